# revision 4
# baseline (speedup 1.0000x reference)
# Trainium2 Bass kernel for dynamic-routing capsule layer (nn_Capsule).
#
# Math (per batch b):
#   u_hat[n,i,j] = sum_d u[n,d] W[d, i*16+j]
#   b=0; for it in 0..2:
#     c = softmax(b, axis=i)
#     o[i,j] = sum_n c[i,n] u_hat[n,i,j]
#     if it<2: o' = l2norm(o); b[i,n] = sum_j o'[i,j] u_hat[n,i,j]
#   out = squash(o)
#
# Key algebraic restructuring (u_hat [B,N,512] never materialized):
#   s[i,d]  = sum_n c[i,n] u[n,d]                  (PE: contract n, u natural)
#   o[i,j]  = (S @ W)[i, i*16+j]                   (PE + mask/group-reduce)
#   vT[d,i] = sum_j W[d,i*16+j] o'[i,j]            (DVE: W * bcast(o'), group-reduce)
#   b[i,n]  = sum_d vT[d,i] u[n,d]                 (PE: contract d, uT transposed)
#
# Tokens are processed in partition-major order n = 32*p + c (contiguous 32KB
# DMA per partition); the order is self-consistent across u/uT/b/e/cT and all
# n-reductions are complete sums, so results are order-invariant.
#
# Big matmuls run in float32r (full PE rate, fp32 rounded to 11 mantissa
# bits => ~1e-4 rel err). All PE inputs are produced as f32r (verifier rule).
#
# Sharding: data-parallel over batch B=32 across 8 cores (4 batches/core),
# W replicated. No collectives.

import numpy as np

N_CORES = 8
B, N, D = 32, 4096, 256
I_CAPS, J_DIM = 32, 16
ROUTINGS = 3
EPS = 1e-7
L2_EPS = 1e-12

F32R = True  # float32r for PE matmuls + transposes (fallback: plain fp32)


def build_nc(b_loc=B // N_CORES, n=N, d=D, enable_asserts=False):
    from contextlib import ExitStack

    import concourse.bass as bass  # noqa: F401
    import concourse.tile as tile
    from concourse import bacc, mybir
    from concourse.masks import make_identity

    f32 = mybir.dt.float32
    pe_dt = mybir.dt.float32r if F32R else f32
    AX = mybir.AxisListType
    OP = mybir.AluOpType
    ACTF = mybir.ActivationFunctionType

    NC = n // 128       # chunks of 128 tokens
    DC = d // 128       # d chunks of 128
    NB = n // 512       # token chunks of 512 (psum bank width)
    IJ = I_CAPS * J_DIM  # 512

    nc = bacc.Bacc("TRN2", target_bir_lowering=False, debug=False,
                   enable_asserts=enable_asserts)
    u_dram = nc.dram_tensor("u", [b_loc, n, d], f32, kind="ExternalInput").ap()
    w_dram = nc.dram_tensor("w", [1, d, IJ], f32, kind="ExternalInput").ap()
    out_dram = nc.dram_tensor("out", [b_loc, I_CAPS, J_DIM], f32,
                              kind="ExternalOutput").ap()

    with tile.TileContext(nc) as tc, ExitStack() as ctx:
        const_pool = ctx.enter_context(tc.tile_pool(name="const", bufs=1))
        u_pool = ctx.enter_context(tc.tile_pool(name="u", bufs=2))
        uT_pool = ctx.enter_context(tc.tile_pool(name="uT", bufs=2))
        cT_pool = ctx.enter_context(tc.tile_pool(name="cT", bufs=2))
        e_pool = ctx.enter_context(tc.tile_pool(name="e", bufs=2))
        small = ctx.enter_context(tc.tile_pool(name="small", bufs=2))
        tiny = ctx.enter_context(tc.tile_pool(name="tiny", bufs=2))
        psum = ctx.enter_context(tc.tile_pool(name="ps", bufs=1, space="PSUM"))

        # ---- constants ----
        ident = const_pool.tile([128, 128], f32, name="ident")
        make_identity(nc, ident[:])
        # f32r-typed identity (produced by a rounding copy => verifier-legal)
        ident_r = const_pool.tile([128, 128], pe_dt, name="ident_r")
        nc.vector.tensor_copy(ident_r[:], ident[:])

        ones_bc = const_pool.tile([I_CAPS, 128], pe_dt, name="ones_bc")
        scratch1 = const_pool.tile([I_CAPS, 128], f32, name="scratch1")
        nc.gpsimd.memset(scratch1[:], 1.0)
        nc.vector.tensor_copy(ones_bc[:], scratch1[:])

        # uniform routing weights for iteration 0 (softmax of zeros == 1/I)
        cT0 = const_pool.tile([128, NC, I_CAPS], pe_dt, name="cT0")
        scratch2 = const_pool.tile([128, NC * I_CAPS], f32, name="scratch2")
        nc.gpsimd.memset(scratch2[:], 1.0 / I_CAPS)
        nc.vector.tensor_copy(cT0[:], scratch2[:].rearrange(
            "p (c i) -> p c i", i=I_CAPS))

        # MASK[i, e] = 1 if e // J_DIM == i else 0   ([32, 512])
        mask = const_pool.tile([I_CAPS, IJ], f32, name="mask")
        nc.gpsimd.memset(mask[:], 0.0)
        # keep 0 where (e - 16*i - 15) > 0, else fill 1  ->  1 iff e <= 16i+15
        nc.gpsimd.affine_select(
            out=mask[:], in_=mask[:], compare_op=OP.is_gt, fill=1.0,
            base=-(J_DIM - 1), pattern=[[1, IJ]], channel_multiplier=-J_DIM)
        # keep where (e - 16*i) >= 0, else fill 0      ->  1 iff 16i <= e <= 16i+15
        nc.gpsimd.affine_select(
            out=mask[:], in_=mask[:], compare_op=OP.is_ge, fill=0.0,
            base=0, pattern=[[1, IJ]], channel_multiplier=-J_DIM)

        # W natural (rounded to f32r by SWDGE cast): w_sb[q, e, f] = W[128e+q, f]
        w_sb = const_pool.tile([128, DC, IJ], pe_dt, name="w_sb")
        nc.gpsimd.dma_start(w_sb[:], w_dram[0].rearrange("(e q) f -> q e f", q=128))

        for b in range(b_loc):
            # ---- load u (f32r cast): u_t[p, c, dd] = u[b, 32p + c, dd] ----
            u_t = u_pool.tile([128, NC, d], pe_dt, tag="u", name=f"u_{b}")
            nc.gpsimd.dma_start(u_t[:],
                                u_dram[b].rearrange("(p c) dd -> p c dd", c=NC))

            # ---- uT[q, e, 128c+p] = u_t[p, c, 128e+q] via PE transposes ----
            uT_t = uT_pool.tile([128, DC, n], pe_dt, tag="uT", name=f"uT_{b}")
            cp_flip = 0
            for e in range(DC):
                for cg in range(0, NC, 4):
                    tr_ps = psum.tile([128, 512], pe_dt, tag="tr", bufs=2,
                                      name=f"trps_{b}_{e}_{cg}")
                    with tc.tile_critical():
                        for k in range(4):
                            c = cg + k
                            nc.tensor.matmul(
                                tr_ps[:, k * 128:(k + 1) * 128],
                                u_t[:, c, e * 128:(e + 1) * 128],
                                ident_r[:],
                                is_transpose=True, start=(k == 0), stop=(k == 3))
                    dst = uT_t[:, e, cg * 128:(cg + 4) * 128]
                    if cp_flip % 2 == 0:
                        nc.vector.tensor_copy(dst, tr_ps[:])
                    else:
                        nc.scalar.copy(dst, tr_ps[:])
                    cp_flip += 1

            cT = cT0
            for it in range(ROUTINGS):
                # ---- matmul1: s[i, dd] = sum_n c[i, n] u[n, dd] ----
                s_ps = psum.tile([I_CAPS, d], f32, tag="s", bufs=1,
                                 name=f"sps_{b}_{it}")
                for c in range(NC):
                    nc.tensor.matmul(s_ps[:], cT[:, c, :], u_t[:, c, :],
                                     start=(c == 0), stop=(c == NC - 1))
                s_sb = small.tile([I_CAPS, d], pe_dt, tag="s_sb",
                                  name=f"ssb_{b}_{it}")
                nc.vector.tensor_copy(s_sb[:], s_ps[:])

                # ---- sT[q, e*32+i] = s[i, 128e+q] ----
                sT_ps = psum.tile([128, DC * I_CAPS], pe_dt, tag="sT", bufs=1,
                                  name=f"sTps_{b}_{it}")
                with tc.tile_critical():
                    for e in range(DC):
                        nc.tensor.matmul(
                            sT_ps[:, e * I_CAPS:(e + 1) * I_CAPS],
                            s_sb[:, e * 128:(e + 1) * 128],
                            ident_r[0:I_CAPS, 0:I_CAPS],
                            is_transpose=True, start=(e == 0), stop=(e == DC - 1))
                sT_sb = small.tile([128, DC * I_CAPS], pe_dt, tag="sT_sb",
                                   name=f"sTsb_{b}_{it}")
                nc.scalar.copy(sT_sb[:], sT_ps[:])

                # ---- O_full = S @ W  [32, 512] ----
                o_ps = psum.tile([I_CAPS, IJ], f32, tag="O", bufs=1,
                                 name=f"Ops_{b}_{it}")
                for e in range(DC):
                    nc.tensor.matmul(o_ps[:], sT_sb[:, e * I_CAPS:(e + 1) * I_CAPS],
                                     w_sb[:, e, :],
                                     start=(e == 0), stop=(e == DC - 1))

                # ---- extract o[i, j] = O_full[i, i*16+j] ----
                om_sb = small.tile([I_CAPS, IJ], f32, tag="om", name=f"om_{b}_{it}")
                nc.vector.tensor_mul(om_sb[:], o_ps[:], mask[:])
                o_sb = tiny.tile([I_CAPS, J_DIM], f32, tag="o", name=f"o_{b}_{it}")
                nc.vector.tensor_reduce(
                    o_sb[:], om_sb[:].rearrange("p (i j) -> p j i", j=J_DIM),
                    axis=AX.X, op=OP.add)

                # ---- ||o||^2 per capsule ----
                sq = tiny.tile([I_CAPS, J_DIM], f32, tag="sq", name=f"sq_{b}_{it}")
                nrm = tiny.tile([I_CAPS, 1], f32, tag="nrm", name=f"nrm_{b}_{it}")
                nc.scalar.activation(sq[:], o_sb[:], ACTF.Square, accum_out=nrm[:])

                if it < ROUTINGS - 1:
                    # ---- l2 normalize scale rr = 1/sqrt(max(nrm, L2_EPS)) ----
                    nrm2 = tiny.tile([I_CAPS, 1], f32, tag="nrm2",
                                     name=f"nrm2_{b}_{it}")
                    nc.vector.tensor_scalar_max(nrm2[:], nrm[:], L2_EPS)
                    rt = tiny.tile([I_CAPS, 1], f32, tag="rt", name=f"rt_{b}_{it}")
                    nc.scalar.sqrt(rt[:], nrm2[:])
                    rr = tiny.tile([I_CAPS, 1], f32, tag="rr", name=f"rr_{b}_{it}")
                    nc.vector.reciprocal(rr[:], rt[:])
                    # masked O scaled by rr -> nonzeros are o'[i,j] at [i, i*16+j]
                    omn = small.tile([I_CAPS, IJ], pe_dt, tag="omn",
                                     name=f"omn_{b}_{it}")
                    nc.vector.tensor_scalar_mul(omn[:], om_sb[:], rr[:, 0:1])

                    # ---- broadcast o'_flat across 128 partitions via ones-matmul:
                    #      E[q, f] = sum_k omn[k, f] = o'[f//16, f%16] ----
                    e_ps = psum.tile([128, IJ], f32, tag="E", bufs=1,
                                     name=f"Eps_{b}_{it}")
                    nc.tensor.matmul(e_ps[:], ones_bc[:], omn[:],
                                     start=True, stop=True)

                    # ---- vT[q, e, i] = sum_j W[128e+q, i*16+j] * o'[i, j] ----
                    vT_sb = tiny.tile([128, DC, I_CAPS], pe_dt, tag="vT",
                                      name=f"vT_{b}_{it}")
                    for e in range(DC):
                        wtmp = small.tile([128, IJ], f32, tag="om",
                                          name=f"wtmp_{b}_{it}_{e}")
                        nc.vector.tensor_mul(wtmp[:], w_sb[:, e, :].bitcast(f32),
                                             e_ps[:])
                        with nc.allow_low_precision(reason="f32r round on store"):
                            nc.vector.tensor_reduce(
                                vT_sb[:, e, :],
                                wtmp[:].rearrange("q (i j) -> q i j", j=J_DIM),
                                axis=AX.X, op=OP.add)

                    # ---- matmul2 + exp: b[i, nn] = sum_d vT[d, i] uT[d, nn] ----
                    e_sb = e_pool.tile([I_CAPS, n], f32, tag="e", name=f"e_{b}_{it}")
                    for k in range(NB):
                        b_ps = psum.tile([I_CAPS, 512], f32, tag="b", bufs=2,
                                         name=f"bps_{b}_{it}_{k}")
                        for e in range(DC):
                            nc.tensor.matmul(b_ps[:], vT_sb[:, e, :],
                                             uT_t[:, e, k * 512:(k + 1) * 512],
                                             start=(e == 0), stop=(e == DC - 1))
                        nc.scalar.activation(e_sb[:, k * 512:(k + 1) * 512], b_ps[:],
                                             ACTF.Exp)

                    # ---- transpose e -> eT blocks, softmax over i -> next cT ----
                    cT = cT_pool.tile([128, NC, I_CAPS], pe_dt, tag="cT",
                                      name=f"cT_{b}_{it + 1}")
                    z_sb = tiny.tile([128, NC], f32, tag="z", name=f"z_{b}_{it}")
                    bpb = 512 // I_CAPS  # transpose blocks per psum bank (16)
                    eT_list = []
                    for g0 in range(0, NC, bpb):
                        bw = min(bpb, NC - g0)
                        eT_ps = psum.tile([128, bw * I_CAPS], f32, tag="tr", bufs=2,
                                          name=f"eTps_{b}_{it}_{g0}")
                        with tc.tile_critical():
                            for t in range(bw):
                                c = g0 + t
                                nc.tensor.matmul(
                                    eT_ps[:, t * I_CAPS:(t + 1) * I_CAPS],
                                    e_sb[:, c * 128:(c + 1) * 128],
                                    ident[0:I_CAPS, 0:I_CAPS],
                                    is_transpose=True,
                                    start=(t == 0), stop=(t == bw - 1))
                        nc.vector.tensor_reduce(
                            z_sb[:, g0:g0 + bw],
                            eT_ps[:].rearrange("q (c i) -> q c i", i=I_CAPS),
                            axis=AX.X, op=OP.add)
                        eT_list.append((g0, bw, eT_ps))
                    r_sb = tiny.tile([128, NC], f32, tag="r", name=f"r_{b}_{it}")
                    nc.vector.reciprocal(r_sb[:], z_sb[:])
                    for g0, bw, eT_ps in eT_list:
                        rb = r_sb[:, g0:g0 + bw]
                        rb = rb.unsqueeze(2).broadcast_to([128, bw, I_CAPS])
                        nc.vector.tensor_mul(
                            cT[:, g0:g0 + bw, :],
                            eT_ps[:].rearrange("q (c i) -> q c i", i=I_CAPS), rb)
                else:
                    # ---- squash: out = sqrt(s2)/(0.5+s2) * o, s2 = nrm + EPS ----
                    s2 = tiny.tile([I_CAPS, 1], f32, tag="s2", name=f"s2_{b}")
                    nc.vector.tensor_scalar_add(s2[:], nrm[:], EPS)
                    rt2 = tiny.tile([I_CAPS, 1], f32, tag="rt2", name=f"rt2_{b}")
                    nc.scalar.sqrt(rt2[:], s2[:])
                    den = tiny.tile([I_CAPS, 1], f32, tag="den", name=f"den_{b}")
                    nc.vector.tensor_scalar_add(den[:], s2[:], 0.5)
                    rden = tiny.tile([I_CAPS, 1], f32, tag="rden", name=f"rden_{b}")
                    nc.vector.reciprocal(rden[:], den[:])
                    scl = tiny.tile([I_CAPS, 1], f32, tag="scl", name=f"scl_{b}")
                    nc.vector.tensor_mul(scl[:], rt2[:], rden[:])
                    o_out = tiny.tile([I_CAPS, J_DIM], f32, tag="oout",
                                      name=f"oout_{b}")
                    nc.vector.tensor_scalar_mul(o_out[:], o_sb[:], scl[:, 0:1])
                    nc.sync.dma_start(out_dram[b], o_out[:])

    nc.compile()
    return nc


_NC_CACHE = {}


def _get_nc():
    if "nc" not in _NC_CACHE:
        _NC_CACHE["nc"] = build_nc()
    return _NC_CACHE["nc"]


def kernel(u_vecs: np.ndarray, W: np.ndarray) -> np.ndarray:
    from concourse.bass_utils import run_bass_kernel_spmd

    u_vecs = np.ascontiguousarray(u_vecs, dtype=np.float32)
    W = np.ascontiguousarray(W, dtype=np.float32)
    b_loc = B // N_CORES
    nc = _get_nc()
    in_maps = [
        {"u": u_vecs[i * b_loc:(i + 1) * b_loc], "w": W}
        for i in range(N_CORES)
    ]
    res = run_bass_kernel_spmd(nc, in_maps, core_ids=list(range(N_CORES)))
    return np.concatenate([r["out"] for r in res.results], axis=0)
